# revision 33
# baseline (speedup 1.0000x reference)
"""AgentImputer Trainium2 kernel.

Contract: kernel(**inputs) takes the FULL unsharded inputs (as produced by
reference.setup_inputs()) and returns the FULL output [64, 40, 2] float32.

Strategy: data-parallel over batch B=64 across 8 NeuronCores (8 batches /
core -> 320 folded LSTM rows per core). Tiny LSTM/GCN weights are
replicated. The 128-step TimeLSTM scan runs feature-major ([hid, row]
tiles) so every matmul contracts along partitions; biases are folded into
the matmuls via a constant-1 row appended to the h/c state; the per-graph
GCN (shared edge_index) becomes dense [40,40] mean-aggregation matmuls.

Perf structure (v3):
- Input marshalling happens on the host: X is embedded (categorical ->
  embedding rows) and transposed to feature-major Xf [W, 71, R] bf16, and
  ts-1 is pre-broadcast to m_bc [W, 100, R] bf16. The device streams one
  [71, R] / [100, R] tile per step with plain 2D DMAs - no on-device
  transposes, no one-hot, no SWDGE broadcast.
- Per step the PE runs 9 matmuls (4 x-side K=71 bf16, 4 h-side K=101
  f32r, 1 Wd f32r); scalar runs 4 ACTIVATEs (tanh, 2x sigmoid, tanh);
  vector runs cadj/t2/c/h; gpsimd runs t1 = cs1*(ts-1) and t3 = i*ct.
"""

import sys

import numpy as np

sys.path.insert(0, "/opt/trn_rl_repo")

# ---------------------------------------------------------------- constants
B, W, N, F_IN = 64, 128, 40, 66
HID = 100
NUM_CONT = 64
NCLS_POS, NCLS_TEAM = 16, 9
EMB_POS, EMB_TEAM = 4, 3
NEW_D = NUM_CONT + EMB_POS + EMB_TEAM  # 71
NCORES = 8
BL = B // NCORES          # 8 local batch elems per core
R = BL * N                # 320 rows per core; row j = 40*b_local + n
KX = NEW_D                # contraction depth for the x-side matmul
G4 = 4 * HID


# ---------------------------------------------------------------- host prep
def _host_weights(inputs):
    f32 = np.float32
    Uall_w = np.asarray(inputs["Uall_w"], f32)       # [400, 71]
    Uall_b = np.asarray(inputs["Uall_b"], f32)       # [400]
    Wall_w = np.asarray(inputs["Wall_w"], f32)       # [400, 100]
    Wall_b = np.asarray(inputs["Wall_b"], f32)       # [400]
    Wd_w = np.asarray(inputs["Wd_w"], f32)           # [100, 100]
    Wd_b = np.asarray(inputs["Wd_b"], f32)           # [100]
    lin_w = np.asarray(inputs["lin_w"], f32)         # [100, 100]
    lin_b = np.asarray(inputs["lin_b"], f32)         # [100]
    edge_index = np.asarray(inputs["edge_index"]).astype(np.int64)  # [2, E]

    # h-side weights with the full gate bias folded in as an extra row
    # (state tiles carry a constant-1 row at partition HID).
    WallT = np.concatenate([Wall_w.T, (Wall_b + Uall_b)[None, :]], 0)  # [101, 400]
    WdT = np.concatenate([Wd_w.T, Wd_b[None, :]], 0)                   # [101, 100]
    linT = np.concatenate([lin_w.T, lin_b[None, :]], 0)                # [101, 100]

    # Mean-aggregation matrix: M[s, d] = count(s->d) / max(deg(d), 1),
    # replicated block-diagonally so one matmul aggregates 3 graphs
    src, dst = edge_index[0], edge_index[1]
    cnt = np.zeros((N, N), f32)
    np.add.at(cnt, (src, dst), 1.0)
    deg = np.maximum(cnt.sum(axis=0), 1.0)
    Mmat = cnt / deg[None, :]
    Mblk = np.zeros((3 * N, 3 * N), f32)
    for i in range(3):
        Mblk[i * N:(i + 1) * N, i * N:(i + 1) * N] = Mmat

    import ml_dtypes
    bf = ml_dtypes.bfloat16
    return {
        "WxT": np.ascontiguousarray(Uall_w.T).astype(bf),                    # [71, 400]
        "WallT": WallT.astype(bf),
        "WdT": WdT,
        "linT": linT.astype(bf),
        "Mblk": np.ascontiguousarray(Mblk, f32),
        "s1l": np.ascontiguousarray(np.asarray(inputs["sage1_l"], f32).T),   # [100, 64]
        "s1r": np.ascontiguousarray(np.asarray(inputs["sage1_r"], f32).T),   # [100, 64]
        "s1b": np.ascontiguousarray(np.asarray(inputs["sage1_lb"], f32)[:, None]),  # [64, 1]
        "s2l": np.ascontiguousarray(np.asarray(inputs["sage2_l"], f32).T),   # [64, 32]
        "s2r": np.ascontiguousarray(np.asarray(inputs["sage2_r"], f32).T),   # [64, 32]
        "s2b": np.ascontiguousarray(np.asarray(inputs["sage2_lb"], f32)[:, None]),  # [32, 1]
        "ow": np.ascontiguousarray(np.asarray(inputs["out_w"], f32).T),      # [32, 2]
        "ob": np.ascontiguousarray(np.asarray(inputs["out_b"], f32)[:, None]),      # [2, 1]
        "hcinit": np.concatenate(
            [np.zeros((HID, R), f32), np.ones((1, R), f32)], 0
        ),  # [101, R]: zero state + constant-1 bias row
        "ident": np.eye(128, dtype=f32),
    }


# ---------------------------------------------------------------- device IR
def build_module(Wsteps=W):
    import concourse.bass as bass
    import concourse.tile as tile
    from concourse import bacc, mybir

    f32 = mybir.dt.float32
    f32r = mybir.dt.float32r
    bf16 = mybir.dt.bfloat16
    AF = mybir.ActivationFunctionType
    PSUM = bass.MemorySpace.PSUM

    def r(ap):
        # float32r view: same 4-byte data, single-pass matmul when N>=256
        return ap.bitcast(f32r)

    nc = bacc.Bacc(
        "TRN2", target_bir_lowering=False, debug=False, num_devices=NCORES
    )

    Xf_in = nc.declare_dram_parameter("Xf", [W, KX, R], bf16, isOutput=False)
    m_in = nc.declare_dram_parameter("m_bc", [W, HID, R], bf16, isOutput=False)
    w_in = {}
    bf16_params = {"WxT", "WallT", "linT"}
    # order matters: the loop below issues the weight DMAs in this order on
    # the gpsimd queue, and the first LSTM step only needs the first four
    for name, shape in [
        ("WdT", [HID + 1, HID]), ("WxT", [KX, G4]), ("WallT", [HID + 1, G4]),
        ("hcinit", [HID + 1, R]),
        ("linT", [HID + 1, HID]), ("Mblk", [3 * N, 3 * N]),
        ("s1l", [HID, 64]), ("s1r", [HID, 64]), ("s1b", [64, 1]),
        ("s2l", [64, 32]), ("s2r", [64, 32]), ("s2b", [32, 1]),
        ("ow", [32, 2]), ("ob", [2, 1]),
        ("ident", [128, 128]),
    ]:
        w_in[name] = nc.declare_dram_parameter(
            name, shape, bf16 if name in bf16_params else f32r, isOutput=False
        )
    # device-natural layout [k, b, n]; host transposes to [b, n, k]
    out_ext = nc.declare_dram_parameter("out", [2, BL, N], f32, isOutput=True)

    with tile.TileContext(nc) as tc:
        with (
            tc.tile_pool(name="consts", bufs=1) as consts,
            tc.tile_pool(name="state", bufs=1) as state,
        ):
            # ---- load constants / weights
            wt = {}
            hT = cT = None
            for name, ext in w_in.items():
                wt[name] = consts.tile(
                    list(ext.shape), ext.dtype, tag=name, name=name
                )
                nc.gpsimd.dma_start(out=wt[name][:], in_=ext[:])
                if name == "hcinit":
                    # persistent state: h/c feature-major with const-1 bias
                    # row (row HID stays 1.0 forever; per-step writes touch
                    # rows 0:HID) -- loaded right after the LSTM weights.
                    # h is bf16 (bounded [-1,1], feeds matmuls only).
                    hT = state.tile([HID + 1, R], bf16, tag="hT")
                    cT = state.tile([HID + 1, R], f32r, tag="cT")
                    nc.vector.tensor_copy(out=hT[:], in_=wt["hcinit"][:].bitcast(f32))
                    nc.gpsimd.dma_start(out=cT[:], in_=w_in["hcinit"][:])

            nodesT = state.tile([HID, R], f32r, tag="nodesT")

            with (
                tc.tile_pool(name="xf", bufs=3) as xf_pool,
                tc.tile_pool(name="mt", bufs=3) as mt_pool,
                tc.tile_pool(name="gsb", bufs=3) as gsb_pool,
                tc.tile_pool(name="work", bufs=3) as work,
                tc.tile_pool(name="pg", bufs=1, space=PSUM) as pg_pool,
                tc.tile_pool(name="pd", bufs=1, space=PSUM) as pd_pool,
                tc.tile_pool(name="pjunk", bufs=1, space=PSUM) as pjunk_pool,
            ):
                # warmer for the PE: junk matmuls that read tnc (written by
                # this step's tanh right before the pre-h-matmul idle), so
                # the scheduler pins them into that idle window and the PE
                # doesn't drop to its low p-state (first matmul after an
                # idle runs ~2x slow). Small N keeps the restart cheap.
                pjunk = pjunk_pool.tile([HID, 512], f32, tag="pjunk")

                def pe_warm(n, rhs):
                    for _ in range(n):
                        nc.tensor.matmul(
                            pjunk[:, 0:rhs.shape[-1]],
                            wt["WallT"][0:HID, 0:HID], rhs,
                            start=True, stop=True,
                        )

                for t in range(Wsteps):
                    # ------- stream this step's x and ts-1 tiles
                    xfT = xf_pool.tile([KX, R], bf16, tag="xfT")
                    nc.sync.dma_start(out=xfT[:], in_=Xf_in[t])
                    mt = mt_pool.tile([HID, R], bf16, tag="mt")
                    nc.sync.dma_start(out=mt[:], in_=m_in[t])

                    # ------- c path: c_adj = c + tanh(Wd@c + bd) * (ts-1)
                    pd = pd_pool.tile([HID, R], f32, tag="pd")
                    nc.tensor.matmul(pd, wt["WdT"][:], cT[:], start=True, stop=True)
                    cs1 = work.tile([HID, R], f32, tag="cs1")
                    nc.scalar.activation(cs1[:], pd, AF.Tanh)
                    t1 = work.tile([HID, R], f32, tag="t1")
                    nc.gpsimd.tensor_mul(t1[:], cs1[:], mt[:])
                    cadj = work.tile([HID, R], bf16, tag="cadj")
                    nc.vector.tensor_add(cadj[:], cT[0:HID, :].bitcast(f32), t1[:])

                    # ------- gates: psum[g] = WxT_g.T @ xfT + WallT_g.T @ h1
                    # split across two psum tiles (pgA double-buffered) so
                    # next step's x-side matmuls can start before sigmoid
                    # consumes the previous gates
                    pgA = pg_pool.tile([HID, 2, 512], f32, tag="pgA", bufs=2)
                    pgB = pg_pool.tile([HID, 2, 512], f32, tag="pgB", bufs=1)
                    halves = (pgA, pgB)
                    # all x-side matmuls first: they only depend on xfT and
                    # psum-buffer availability, so the in-order PE queue can
                    # retire them during the previous step's tail instead of
                    # head-of-line-blocking the h-side matmuls behind hmul
                    for g in range(4):
                        nc.tensor.matmul(
                            halves[g // 2][:, g % 2, 0:R],
                            wt["WxT"][:, HID * g:HID * (g + 1)],
                            xfT[:], start=True, stop=False,
                        )
                    for g in range(4):
                        nc.tensor.matmul(
                            halves[g // 2][:, g % 2, 0:R],
                            wt["WallT"][:, HID * g:HID * (g + 1)],
                            hT[:], start=False, stop=True,
                        )
                    gs = gsb_pool.tile([HID, 4, R], bf16, tag="gs")
                    nc.scalar.activation(gs[:, 0:2, :], pgA[:, :, 0:R], AF.Sigmoid)
                    nc.scalar.activation(gs[:, 2:4, :], pgB[:, :, 0:R], AF.Sigmoid)

                    # ------- state update: c = f*c_adj + i*ct ; h = o*tanh(c)
                    t2 = work.tile([HID, R], bf16, tag="t2")
                    nc.vector.tensor_mul(t2[:], gs[:, 0, :], cadj[:])
                    t3 = work.tile([HID, R], bf16, tag="t3")
                    nc.vector.tensor_mul(t3[:], gs[:, 1, :], gs[:, 3, :])
                    nc.vector.tensor_add(cT[0:HID, :], t2[:], t3[:])
                    tnc = work.tile([HID, R], bf16, tag="tnc")
                    nc.scalar.activation(tnc[:], cT[0:HID, :].bitcast(f32), AF.Tanh)
                    pe_warm(4, tnc[:, 0:80])
                    nc.vector.tensor_mul(hT[0:HID, :], gs[:, 2, :], tnc[:])

                # ---- output linear: nodes = relu(lin @ h + lb)
                pl = pd_pool.tile([HID, R], f32, tag="pd")
                nc.tensor.matmul(pl, wt["linT"][:], hT[:], start=True, stop=True)
                nc.scalar.activation(nodesT[:], pl, AF.Relu)

            # ---------------- GCN: two SAGE layers + output proj
            with (
                tc.tile_pool(name="gc", bufs=3) as gc,
                tc.tile_pool(name="gcs", bufs=1) as gcs,
                tc.tile_pool(name="gp", bufs=2, space=PSUM) as gp,
                tc.tile_pool(name="gp1", bufs=1, space=PSUM) as gp1,
            ):
                GRP = [(0, 120), (120, 120), (240, 80)]

                def mean_agg(srcT, hid):
                    """srcT: [hid, R] feature-major -> aggT [hid, R].

                    Aggregates 3 graphs per matmul via the block-diagonal
                    mean matrix (edge_index is shared across the batch).
                    """
                    aggT = gcs.tile([hid, R], f32r, tag=f"agg{hid}", name="aggT")
                    for off, rows in GRP:
                        ptr = gp.tile([128, 128], f32, tag="ptr")
                        nc.tensor.transpose(
                            r(ptr[0:rows, 0:hid]),
                            srcT[:, off:off + rows],
                            wt["ident"][:hid, :hid],
                        )
                        nbm = gc.tile([128, 128], f32r, tag="nbm")
                        nc.any.tensor_copy(
                            out=nbm[0:rows, 0:hid], in_=ptr[0:rows, 0:hid]
                        )
                        pa = gp.tile([128, 128], f32, tag="pa")
                        nc.tensor.matmul(
                            pa[0:hid, 0:rows],
                            nbm[0:rows, 0:hid],
                            wt["Mblk"][0:rows, 0:rows],
                            start=True, stop=True,
                        )
                        nc.any.tensor_copy(
                            out=aggT[:, off:off + rows], in_=pa[0:hid, 0:rows]
                        )
                    return aggT

                agg1 = mean_agg(nodesT, HID)
                pg1 = gp1.tile([64, R], f32, tag="pg1")
                nc.tensor.matmul(pg1, wt["s1l"][:], agg1[:], start=True, stop=False)
                nc.tensor.matmul(pg1, wt["s1r"][:], nodesT[:], start=False, stop=True)
                g1T = gcs.tile([64, R], f32r, tag="g1T")
                nc.scalar.activation(g1T[:], pg1, AF.Relu, bias=wt["s1b"][:].bitcast(f32))

                agg2 = mean_agg(g1T, 64)
                pg2 = gp1.tile([32, R], f32, tag="pg2")
                nc.tensor.matmul(pg2, wt["s2l"][:], agg2[:], start=True, stop=False)
                nc.tensor.matmul(pg2, wt["s2r"][:], g1T[:], start=False, stop=True)
                g2T = gcs.tile([32, R], f32r, tag="g2T")
                nc.scalar.activation(g2T[:], pg2, AF.Relu, bias=wt["s2b"][:].bitcast(f32))

                po = gp1.tile([2, R], f32, tag="po")
                nc.tensor.matmul(po, wt["ow"][:], g2T[:], start=True, stop=True)
                oT = gcs.tile([2, R], f32, tag="oT")
                nc.scalar.activation(oT[:], po, AF.Relu, bias=wt["ob"][:].bitcast(f32))

                nc.sync.dma_start(
                    out=out_ext.rearrange("k b n -> k (b n)"), in_=oT[:]
                )

    nc.compile()
    return nc


# ---------------------------------------------------------------- execution
_CACHE = {}


def _get_module():
    if "nc" not in _CACHE:
        _CACHE["nc"] = build_module()
    return _CACHE["nc"]


def _host_x(inputs):
    """Embed categoricals and transpose to feature-major [W, 71, B*N]."""
    f32 = np.float32
    import ml_dtypes
    bf = ml_dtypes.bfloat16
    X = np.asarray(inputs["X"], f32)                 # [B, W, N, 66]
    emb_pos = np.asarray(inputs["emb_pos"], f32)
    emb_team = np.asarray(inputs["emb_team"], f32)

    def embed(idx, table):
        mask = idx > 0
        safe = np.where(mask, idx - 1, 0)
        return table[safe] * mask[..., None].astype(f32)

    ip = X[..., 64].astype(np.int64)
    it = X[..., 65].astype(np.int64)
    Xe = np.concatenate(
        [X[..., :NUM_CONT], embed(ip, emb_pos), embed(it, emb_team)], -1
    )                                                # [B, W, N, 71]
    # -> [B, W, 71, N] -> per-core [W, 71, BL, N] -> [W, 71, R]
    return np.ascontiguousarray(Xe.transpose(0, 1, 3, 2)).astype(bf)


def make_in_maps(inputs):
    f32 = np.float32
    import ml_dtypes
    bf = ml_dtypes.bfloat16
    Xe = _host_x(inputs)                             # [B, W, 71, N] bf16
    ts = np.asarray(inputs["ts_list"], f32)
    wts = _host_weights(inputs)
    in_maps = []
    for c in range(NCORES):
        Xc = Xe[c * BL:(c + 1) * BL]                 # [BL, W, 71, N]
        Xf = np.ascontiguousarray(
            Xc.transpose(1, 2, 0, 3).reshape(W, KX, R)
        )
        tsl = ts[c * BL:(c + 1) * BL]                # [BL, W, N]
        tsa = tsl.transpose(1, 0, 2).reshape(W, R) - 1.0
        m_bc = np.ascontiguousarray(
            np.broadcast_to(tsa[:, None, :], (W, HID, R)).astype(bf)
        )
        m = {"Xf": Xf, "m_bc": m_bc}
        m.update(wts)
        in_maps.append(m)
    return in_maps


def kernel(**inputs) -> np.ndarray:
    from concourse.bass_utils import run_bass_kernel_spmd

    nc = _get_module()
    in_maps = make_in_maps(inputs)
    res = run_bass_kernel_spmd(nc, in_maps, list(range(NCORES)))
    outs = [
        np.transpose(res.results[c]["out"], (1, 2, 0)) for c in range(NCORES)
    ]
    return np.ascontiguousarray(np.concatenate(outs, axis=0).astype(np.float32))
